# revision 12
# baseline (speedup 1.0000x reference)
"""DGI (Deep Graph Infomax) forward kernel for 8 TRN2 NeuronCores.

Problem (all shapes hardcoded):
  seq1, seq2: [1, 8192, 128] f32   node features
  adj:        [1, 8192, 8192] f32  dense adjacency
  cc_label:   [8, 1024] i32        community partition (arange layout)
  W: [128,128], b: [128], Wb: [128,128], bb: [] f32
  out:        [1, 16384] f32       = concat(ret1, ret2)

Math per GCN branch: h = relu(adj @ (seq @ W) + b), reassociated to
(adj @ seq) @ W so the big contraction uses seq tiles as the stationary
operand and a host-transposed adj block as the moving operand. Everything
lives in "transposed" space (features on partitions): the community mean
is a free-axis accumulation and the bilinear scores are 1-column matmuls.

Sharding: core k owns nodes [1024k, 1024k+1024) == community k (cc_label
is arange). No collectives.

Data layout (the whole point of this version): seq tile t is consumed by
exactly the two matmuls of m-tile t, so the host packs, per partition p
and tile t, [seq1 row | seq2 row | adjT rows] into one combined tensor
comb[128, 64, 1280] f16 (2560 B per partition-tile). One DMA stream in
tile order IS the consumption order, every transfer has >=5 KB contiguous
per-partition runs (small runs were the previous bottleneck: 512 B
packets move at ~20 GB/s/engine vs 4 KB at full rate), and no persistent
seq buffer is needed.

Schedule:
  - sync HWDGE queue: comb[0:2][2:4][4:6][6:8][8:12][12:16] into a warm
    tile (fine-grained so the PE starts as early as possible), then
    8-tile groups [16:24][32:40][48:56][56:64] from a 3-buffer pool.
  - scalar HWDGE queue: params, then groups [24:32][40:48] (it starts
    ~1 us later than sync; interleaving two queues overlaps the
    small-packet warmup with steady streaming).
  - All small matmuls (W-contraction, cw, scores) in fp16. adj is
    pre-scaled by 256 (fp16 range); the relu computes h'=relu(z+256b)
    and the 1/256 is folded into host-side wbt and the sigmoid scale.
  - m-dim split in halves of 32 tiles; half-1's W-contraction + copies
    run mid-stream. For the last 16 m-tiles, branch 0 (which gates
    sigmoid -> cw -> all scores) streams before branch 1, and branch 0's
    epilogue PE ops are interleaved between branch-1 matmul batches (the
    PE queue is in-order). Branch 1 finishes c-major over the last 3
    tiles so chunk 0's epilogue overlaps chunk 1's matmuls.
  - Output via a single DMA on the sync queue.
"""

import numpy as np

import concourse.bass as bass
import concourse.tile as tile
from concourse import bacc, mybir
from concourse.bass_utils import run_bass_kernel_spmd

N = 8192          # nodes
D = 128           # input feature dim
H = 128           # hidden dim
NC = 8            # communities / cores
CS = N // NC      # community size (nodes per core)
MT = N // 128     # number of 128-row m-tiles (64)
HALF = MT // 2
CHUNK = 512       # matmul moving free dim (psum bank width in fp32)
NCH = CS // CHUNK # n-chunks per core (2)

F32 = mybir.dt.float32
F16 = mybir.dt.float16
ADJ_SCALE = 256.0  # keeps fp16(adj*scale) in the normal range; undone via
                   # host-prescaled wbt (scores) and the sigmoid scale (mean)

ROW = 2 * D + CS   # per-partition-tile row: seq1 | seq2 | adjT = 1280 f16
AOFF = 2 * D       # adj offset within a row

WARM_NT = 16
WARM_RANGES = [(0, 2), (2, 4), (4, 6), (6, 8), (8, 12), (12, 16)]
SYNC_GROUPS = [(16, 8), (32, 8), (48, 8), (56, 8)]
SCALAR_GROUPS = [(24, 8), (40, 8)]
TAIL_T0 = 48      # last 16 m-tiles stream branch 0 fully before branch 1


def _build_module() -> bass.Bass:
    nc = bacc.Bacc()

    comb = nc.declare_dram_parameter("comb", [128, MT, ROW], F16, isOutput=False)
    w = nc.declare_dram_parameter("w", [D, H], F16, isOutput=False)
    wbt = nc.declare_dram_parameter("wbt", [H, H], F16, isOutput=False)
    bvec = nc.declare_dram_parameter("bvec", [H, 1], F32, isOutput=False)
    bbvec = nc.declare_dram_parameter("bbvec", [1, 1], F32, isOutput=False)
    out = nc.declare_dram_parameter("out", [2, CS], F32, isOutput=True)

    with tile.TileContext(nc) as tc:
        _emit(tc, comb, w, wbt, bvec, bbvec, out)
    nc.finalize()
    return nc


def _emit(tc, comb, w, wbt, bvec, bbvec, out):
    nc = tc.nc
    AF = mybir.ActivationFunctionType
    with (
        tc.tile_pool(name="singles", bufs=1) as singles,
        tc.tile_pool(name="adj_pool", bufs=1) as adj_pool,
        tc.tile_pool(name="misc", bufs=1) as misc,
        tc.tile_pool(name="psum", bufs=1, space="PSUM") as psum,
    ):
        # ---- DMA program (per-queue FIFO == consumption order).
        warm_sb = singles.tile([128, WARM_NT, ROW], F16)
        adj_bufs = {t: (warm_sb, t) for t in range(WARM_NT)}
        for t0, t1 in WARM_RANGES:
            nc.sync.dma_start(out=warm_sb[:, t0:t1], in_=comb[:, t0:t1])

        group_order = sorted(
            [(t0, gn, "sync") for t0, gn in SYNC_GROUPS]
            + [(t0, gn, "scalar") for t0, gn in SCALAR_GROUPS]
        )
        w_sb = singles.tile([D, H], F16)
        nc.scalar.dma_start(out=w_sb, in_=w[:])
        wbt_sb = singles.tile([H, H], F16)
        nc.scalar.dma_start(out=wbt_sb, in_=wbt[:])
        b_sb = singles.tile([H, 1], F32)
        nc.scalar.dma_start(out=b_sb, in_=bvec[:])
        bb_sb = singles.tile([1, 1], F32)
        nc.scalar.dma_start(out=bb_sb, in_=bbvec[:])
        for t0, gn, q in group_order:
            buf = adj_pool.tile([128, gn, ROW], F16, name="adj_sb",
                                tag="adj_sb", bufs=3)
            eng = nc.sync if q == "sync" else nc.scalar
            eng.dma_start(out=buf, in_=comb[:, t0:t0 + gn])
            for u in range(gn):
                adj_bufs[t0 + u] = (buf, u)

        # ---- Tiles.
        z = [
            [
                [psum.tile([128, CHUNK], F32, name=f"z_{h}_{s}_{c}") for c in range(NCH)]
                for s in range(2)
            ]
            for h in range(2)
        ]
        zt = [
            [
                [misc.tile([128, CHUNK], F16, name=f"zt_{h}_{s}_{c}") for c in range(NCH)]
                for s in range(2)
            ]
            for h in range(2)
        ]
        h_sb = [
            [misc.tile([128, CHUNK], F16, name=f"h_{s}_{c}") for c in range(NCH)]
            for s in range(2)
        ]
        csum = [misc.tile([H, 1], F32, name=f"csum_{c}") for c in range(NCH)]
        csum_tot = misc.tile([H, 1], F32)
        c_sb = misc.tile([H, 1], F16)
        cw_sb = misc.tile([H, 1], F16)
        out_sb = misc.tile([1, 2, CS], F32)

        def mm(t, s, cs=(0, 1)):
            buf, u = adj_bufs[t]
            lhsT = buf[:, u, s * D:(s + 1) * D]
            for c in cs:
                nc.tensor.matmul(
                    z[t // HALF][s][c],
                    lhsT,
                    buf[:, u, AOFF + c * CHUNK:AOFF + (c + 1) * CHUNK],
                    start=(t % HALF == 0),
                    stop=(t % HALF == HALF - 1),
                )

        def copy_z(h, s):
            # psum fp32 -> sbuf fp16, chunk 0 on vector / chunk 1 on scalar
            nc.vector.tensor_copy(out=zt[h][s][0], in_=z[h][s][0])
            nc.scalar.activation(out=zt[h][s][1], in_=z[h][s][1], func=AF.Copy)

        def wagg(h, s, start, stop):
            for c in range(NCH):
                nc.tensor.matmul(z[0][s][c], w_sb, zt[h][s][c], start=start, stop=stop)

        # ---- Main stream.
        for t in range(TAIL_T0):
            for s in range(2):
                mm(t, s)
            if t == HALF - 1:
                for s in range(2):
                    copy_z(0, s)
            if t == HALF + 15:
                for s in range(2):
                    wagg(0, s, start=True, stop=False)

        # Last 16 tiles: see module docstring.
        for t in range(TAIL_T0, MT):
            mm(t, 0)
        copy_z(1, 0)
        for t in range(TAIL_T0, TAIL_T0 + 4):
            mm(t, 1)
        wagg(1, 0, start=False, stop=True)
        for c in range(NCH):
            nc.scalar.activation(
                out=h_sb[0][c],
                in_=z[0][0][c],
                func=AF.Relu,
                bias=b_sb,
                accum_out=csum[c],
            )
        nc.vector.tensor_add(out=csum_tot, in0=csum[0], in1=csum[1])
        nc.scalar.activation(
            out=c_sb, in_=csum_tot, func=AF.Sigmoid, scale=1.0 / (CS * ADJ_SCALE)
        )
        for t in range(TAIL_T0 + 4, TAIL_T0 + 9):
            mm(t, 1)
        cw_ps = z[1][0][0]
        nc.tensor.matmul(cw_ps[:, :1], wbt_sb, c_sb, start=True, stop=True)
        nc.vector.tensor_copy(out=cw_sb, in_=cw_ps[:, :1])
        for t in range(TAIL_T0 + 9, TAIL_T0 + 11):
            mm(t, 1)
        # branch-0 scores into banks freed by the branch-0 copies/relu
        sc0 = [z[1][0][1], z[0][0][0]]
        for c in range(NCH):
            nc.tensor.matmul(sc0[c][:1, :], cw_sb, h_sb[0][c], start=True, stop=True)
        nc.vector.tensor_scalar_add(
            out=out_sb[:, 0, 0:CHUNK], in0=sc0[0][:1, :], scalar1=bb_sb
        )
        nc.scalar.activation(
            out=out_sb[:, 0, CHUNK:], in_=sc0[1][:1, :], func=AF.Identity, bias=bb_sb
        )
        for t in range(TAIL_T0 + 11, MT - 3):
            mm(t, 1)
        for t in range(MT - 3, MT):
            mm(t, 1, cs=(0,))
        nc.vector.tensor_copy(out=zt[1][1][0], in_=z[1][1][0])
        for t in range(MT - 3, MT):
            mm(t, 1, cs=(1,))
        nc.tensor.matmul(z[0][1][0], w_sb, zt[1][1][0], start=False, stop=True)
        nc.scalar.activation(out=zt[1][1][1], in_=z[1][1][1], func=AF.Copy)
        nc.vector.tensor_scalar(
            out=h_sb[1][0],
            in0=z[0][1][0],
            scalar1=b_sb,
            scalar2=0.0,
            op0=mybir.AluOpType.add,
            op1=mybir.AluOpType.max,
        )
        nc.tensor.matmul(z[0][1][1], w_sb, zt[1][1][1], start=False, stop=True)
        sc1 = [z[1][1][0], z[1][1][1]]
        nc.tensor.matmul(sc1[0][:1, :], cw_sb, h_sb[1][0], start=True, stop=True)
        nc.scalar.activation(
            out=h_sb[1][1], in_=z[0][1][1], func=AF.Relu, bias=b_sb
        )
        nc.vector.tensor_scalar_add(
            out=out_sb[:, 1, 0:CHUNK], in0=sc1[0][:1, :], scalar1=bb_sb
        )
        nc.tensor.matmul(sc1[1][:1, :], cw_sb, h_sb[1][1], start=True, stop=True)
        nc.scalar.activation(
            out=out_sb[:, 1, CHUNK:], in_=sc1[1][:1, :], func=AF.Identity, bias=bb_sb
        )
        nc.sync.dma_start(out=out[:, :].unsqueeze(0), in_=out_sb)


_MODULE_CACHE: list = []


def get_module() -> bass.Bass:
    if not _MODULE_CACHE:
        _MODULE_CACHE.append(_build_module())
    return _MODULE_CACHE[0]


def shard_inputs(inputs: dict) -> list[dict]:
    """Full inputs -> per-core input maps (row-block sharding of adjT).

    comb[p, t, :] = [seq1[128t+p, :] | seq2[128t+p, :] | adjT rows] (f16),
    adj pre-scaled by 256; wbt = Wb.T/256; bvec = 256*b (see module doc).
    """
    s1 = np.asarray(inputs["seq1"], np.float32)[0].astype(np.float16)
    s2 = np.asarray(inputs["seq2"], np.float32)[0].astype(np.float16)
    seq_part = np.stack([s1, s2], axis=0).reshape(2, MT, 128, D).transpose(2, 1, 0, 3)
    seq_part = seq_part.reshape(128, MT, 2 * D)
    adj16 = (np.asarray(inputs["adj"], np.float32)[0] * ADJ_SCALE).astype(np.float16)
    w = np.asarray(inputs["W"], np.float32).astype(np.float16)
    wbt = np.ascontiguousarray(
        (np.asarray(inputs["Wb"], np.float32).T / ADJ_SCALE).astype(np.float16)
    )
    bvec = (np.asarray(inputs["b"], np.float32) * ADJ_SCALE).reshape(H, 1).copy()
    bbvec = np.asarray(inputs["bb"], np.float32).reshape(1, 1).copy()

    in_maps = []
    for k in range(NC):
        adjt = adj16[k * CS:(k + 1) * CS, :].T.reshape(MT, 128, CS).transpose(1, 0, 2)
        comb = np.empty((128, MT, ROW), np.float16)
        comb[:, :, :2 * D] = seq_part
        comb[:, :, 2 * D:] = adjt
        in_maps.append(
            {
                "comb": comb,
                "w": w,
                "wbt": wbt,
                "bvec": bvec,
                "bbvec": bbvec,
            }
        )
    return in_maps


def gather_output(core_outs: list[np.ndarray], cc_label: np.ndarray) -> np.ndarray:
    """Per-core [2, CS] score blocks -> full [1, 2N] output.

    Scatter through cc_label mirrors the reference's .at[flat].set: entry
    (community k, position j) is the score of node cc_label[k, j].
    """
    sc1 = np.concatenate([o[0] for o in core_outs]).astype(np.float32)
    sc2 = np.concatenate([o[1] for o in core_outs]).astype(np.float32)
    flat = np.asarray(cc_label).reshape(-1)
    ret1 = np.zeros(N, np.float32)
    ret2 = np.zeros(N, np.float32)
    ret1[flat] = sc1
    ret2[flat] = sc2
    return np.concatenate([ret1, ret2])[None, :]


def kernel(**inputs) -> np.ndarray:
    nc = get_module()
    in_maps = shard_inputs(inputs)
    res = run_bass_kernel_spmd(nc, in_maps, core_ids=list(range(NC)))
    core_outs = [res.results[k]["out"] for k in range(NC)]
    return gather_output(core_outs, inputs["cc_label"])


if __name__ == "__main__":
    nc = get_module()
    print("module built ok")
